# revision 12
# baseline (speedup 1.0000x reference)
"""MoE FFN (16 experts, top-2, SwiGLU, + shared expert) on 8 trn2 NeuronCores.

Strategy (expert-parallel, per sharding hint):
  - Host computes the (tiny) router in fp64, dispatches tokens by topk_idx:
    each core c owns experts {slot0[c], slot1[c]} and receives its experts'
    tokens gathered + transposed into [feature, token] layout, capacity-padded.
  - Device runs the heavy compute in bf16: per expert gate/up projections,
    SwiGLU, down projection, scaled by the top-2 softmax combine weight.
  - Shared expert is token-parallel: core c processes tokens [512c, 512c+512)
    with the full (replicated) shared weights.
  - Host scatter-adds per-expert outputs back by token index (the "unshard")
    and adds the shared-expert shard outputs. No on-device collectives.

Schedule notes (what makes this fast):
  - 8 warmup matmuls on a memset tile run first so the PE HAM clock gate
    (cold 1.2 GHz -> warm 2.4 GHz after ~3.4us of activity) opens during the
    initial DMA ramp instead of during real work.
  - Startup loads are striped across the sync/gpsimd/scalar/vector DMA queues
    so the first expert's activations+weights land in ~3us (vs ~16us if
    serialized on two queues).
  - All SBUF tensors are distinct static allocations (no WAR hazards); PSUM
    rotates through a single 8-bank pool for max matmul ILP.
  - Outputs stream out in bf16 as soon as each down-projection tile finishes.
"""

import math
import os
import sys

for _p in ("/opt/trn_rl_repo", "/root/.axon_site", "/root/.axon_site/_ro/trn_rl_repo",
           "/root/.axon_site/_ro/pypackages"):
    if os.path.isdir(_p) and _p not in sys.path:
        sys.path.append(_p)

import numpy as np

# The agent image's `antenv` package lacks `axon_hooks`, which
# concourse.bass_utils imports when BASS_TRACE=1. Install a compatible
# shim (and register the real NTFF hook if the axon .so is present) so
# tracing works and trace=True doesn't crash.
try:
    from antenv import axon_hooks as _ah  # noqa: F401
except ImportError:
    try:
        import types

        import antenv as _antenv

        _ah = types.ModuleType("antenv.axon_hooks")
        _ah._hook = None
        _ah.set_axon_ntff_profile_hook = lambda h: setattr(_ah, "_hook", h)
        _ah.get_axon_ntff_profile_hook = lambda: _ah._hook
        sys.modules["antenv.axon_hooks"] = _ah
        _antenv.axon_hooks = _ah
        try:
            from trn_agent_boot.trn_boot import _ntff_profile_via_ctypes

            if os.path.exists("/opt/axon/libaxon_pjrt.so"):
                _ah._hook = _ntff_profile_via_ctypes("/opt/axon/libaxon_pjrt.so")
        except Exception:
            pass
    except Exception:
        pass

import ml_dtypes

DIM = 1024
ED = 512          # expert hidden dim
E = 16            # experts
TOPK = 2
SH = 1024         # shared expert hidden dim
N_CORES = 8
P = 128
DD = DIM // P     # 8 feature chunks
HE = ED // P      # 4 expert-hidden chunks
HS = SH // P      # 8 shared-hidden chunks
TS = (4 * 1024) // N_CORES  # 512 shared-expert tokens per core

_BF = ml_dtypes.bfloat16

# compiled-program cache keyed by capacity (C0, C1)
_PROGRAMS = {}
LAST_RESULT = None  # BassKernelResults of the most recent run (for test.py)

N_WARMUP_MM = 8


def _token_tiles(C):
    """Split C into matmul free-dim tiles <= 512 (one PSUM bank)."""
    n_t = math.ceil(C / 512)
    base, rem = divmod(C, n_t)
    sizes = [base + (1 if i < rem else 0) for i in range(n_t)]
    tiles, off = [], 0
    for sz in sizes:
        tiles.append((off, sz))
        off += sz
    return tiles


def _build_program(C0, C1):
    import concourse.bacc as bacc
    import concourse.mybir as mybir
    import concourse.tile as tile

    f32 = mybir.dt.float32
    bf16 = mybir.dt.bfloat16
    SIG = mybir.ActivationFunctionType.Silu

    nc = bacc.Bacc("TRN2", target_bir_lowering=False, debug=False)

    CS = [C0, C1]
    # ---- DRAM I/O (per-core) ----
    # gathered tokens, transposed: xg{s}[p, dd, t] = x[idx_e[t], dd*128+p]
    xg_ds = [nc.dram_tensor(f"xg{i}", [P, DD, CS[i]], bf16, kind="ExternalInput")
             for i in range(2)]
    # combine weights pre-broadcast over partitions: bc{s}[p, t] = w_e[t]
    bc_ds = [nc.dram_tensor(f"bc{i}", [P, CS[i]], bf16, kind="ExternalInput")
             for i in range(2)]
    # expert weights: wg{s}/wu{s}[p, hh, dd, c] = W[dd*128+p, hh*128+c]
    wg_ds = [nc.dram_tensor(f"wg{i}", [P, HE, DD, P], bf16, kind="ExternalInput")
             for i in range(2)]
    wu_ds = [nc.dram_tensor(f"wu{i}", [P, HE, DD, P], bf16, kind="ExternalInput")
             for i in range(2)]
    # down weights: wd{s}[p, o, j, c] = down[j*128+p, o*128+c]
    wd_ds = [nc.dram_tensor(f"wd{i}", [P, DD, HE, P], bf16, kind="ExternalInput")
             for i in range(2)]
    # shared-expert token shard, transposed like xg
    xs_d = nc.dram_tensor("xs", [P, DD, TS], bf16, kind="ExternalInput")
    # shared gate/up: [p, hh, dd, c] = Wsh.T[dd*128+p, hh*128+c]
    wgs_d = nc.dram_tensor("wgs", [P, HS, DD, P], bf16, kind="ExternalInput")
    wus_d = nc.dram_tensor("wus", [P, HS, DD, P], bf16, kind="ExternalInput")
    # shared down: sd[p, o, j, c] = sh_down.T[j*128+p, o*128+c]
    sd_d = nc.dram_tensor("sd", [P, DD, HS, P], bf16, kind="ExternalInput")

    # outputs (bf16): ye{s}[o, p, t] = (expert out)[d=o*128+p, token t] * combine
    ye_ds = [nc.dram_tensor(f"ye{i}", [DD, P, CS[i]], bf16, kind="ExternalOutput")
             for i in range(2)]
    ys_d = nc.dram_tensor("ys", [DD, P, TS], bf16, kind="ExternalOutput")

    tiless = [_token_tiles(C0), _token_tiles(C1)]

    # round-robin DMA issue queues for bulk loads / stores
    def q_cycle(engines):
        i = 0
        while True:
            yield engines[i % len(engines)]
            i += 1

    with tile.TileContext(nc) as tc:
        with (
            tc.tile_pool(name="sb", bufs=1) as sb,
            tc.tile_pool(name="work", bufs=1) as work,
            tc.tile_pool(name="psum", bufs=1, space="PSUM") as psum,
        ):
            def pp():
                return psum.tile([P, 512], f32, tag="pp", name="pp", bufs=8)

            # ---- warmup: open the HAM clock gate while DMAs ramp ----
            warm = sb.tile([P, 512], bf16, tag="warm", name="warm")
            with tc.high_priority():
                nc.vector.memset(warm[:], 0.0)
                for _ in range(N_WARMUP_MM):
                    nc.tensor.matmul(pp(), warm[:, 0:P], warm[:], start=True,
                                     stop=True)

            # ---- static SBUF allocations ----
            xg_sbs = [sb.tile([P, DD, CS[i]], bf16, tag=f"xg{i}", name="xg")
                      for i in range(2)]
            bc_sbs = [sb.tile([P, CS[i]], bf16, tag=f"bc{i}", name="bc")
                      for i in range(2)]
            wg_sbs = [sb.tile([P, HE, DD, P], bf16, tag=f"wg{i}", name="wg")
                      for i in range(2)]
            wu_sbs = [sb.tile([P, HE, DD, P], bf16, tag=f"wu{i}", name="wu")
                      for i in range(2)]
            wd_sbs = [sb.tile([P, DD, HE, P], bf16, tag=f"wd{i}", name="wd")
                      for i in range(2)]
            hT_sbs = [sb.tile([P, HE, CS[i]], bf16, tag=f"hT{i}", name="hT")
                      for i in range(2)]
            xs_sb = sb.tile([P, DD, TS], bf16, tag="xs", name="xs")
            wgs_sb = sb.tile([P, HS, DD, P], bf16, tag="wgs", name="wgs")
            wus_sb = sb.tile([P, HS, DD, P], bf16, tag="wus", name="wus")
            sd_sb = sb.tile([P, DD, HS, P], bf16, tag="sd", name="sd")
            sT_sb = sb.tile([P, HS, TS], bf16, tag="sT", name="sT")

            # ---- load issue, in consumption order ----
            # IMPORTANT: all DMAs writing one tile must share a ring — Tile
            # serializes same-tile writes issued from different rings (each
            # waits for the previous chunk's completion). Parallelism comes
            # from putting different tiles on different rings.
            #   sync / scalar: fast hardware-DGE rings (expert-phase data)
            #   gpsimd: software DGE (slower) — shared-expert stream
            # scalar ring: activations first (first compute needs xg0 tile A)
            for d in range(DD):
                nc.scalar.dma_start(xg_sbs[0][:, d], xg_ds[0][:, d])
            nc.scalar.dma_start(bc_sbs[0][:], bc_ds[0][:])
            nc.scalar.dma_start(bc_sbs[1][:], bc_ds[1][:])
            for d in range(0, DD, 2):
                nc.scalar.dma_start(xg_sbs[1][:, d:d + 2], xg_ds[1][:, d:d + 2])
            for o in range(0, DD, 2):
                nc.scalar.dma_start(wd_sbs[1][:, o:o + 2], wd_ds[1][:, o:o + 2])
            # sync ring: expert gate/up weights in consumption order
            for hh in range(HE):
                nc.sync.dma_start(wg_sbs[0][:, hh], wg_ds[0][:, hh])
                nc.sync.dma_start(wu_sbs[0][:, hh], wu_ds[0][:, hh])
            for hh in range(HE):
                nc.sync.dma_start(wg_sbs[1][:, hh], wg_ds[1][:, hh])
                nc.sync.dma_start(wu_sbs[1][:, hh], wu_ds[1][:, hh])
            for o in range(0, DD, 2):
                nc.sync.dma_start(wd_sbs[0][:, o:o + 2], wd_ds[0][:, o:o + 2])
            # gpsimd ring: shared-expert stream, deadline order
            for hh in range(2):
                nc.gpsimd.dma_start(wgs_sb[:, hh], wgs_d[:, hh])
                nc.gpsimd.dma_start(wus_sb[:, hh], wus_d[:, hh])
            for d in range(0, DD, 2):
                nc.gpsimd.dma_start(xs_sb[:, d:d + 2], xs_d[:, d:d + 2])
            for hh in range(2, HS):
                nc.gpsimd.dma_start(wgs_sb[:, hh], wgs_d[:, hh])
                nc.gpsimd.dma_start(wus_sb[:, hh], wus_d[:, hh])
            for o in range(0, DD, 2):
                nc.gpsimd.dma_start(sd_sb[:, o:o + 2], sd_d[:, o:o + 2])

            # ---- compute phases ----
            def gu_phase(n_h, wg_sb, wu_sb, x_sb, hT, toks):
                for (t0, tsz) in toks:
                    for hh in range(n_h):
                        pg = pp()[:, :tsz]
                        pu = pp()[:, :tsz]
                        for d in range(DD):
                            nc.tensor.matmul(pg, wg_sb[:, hh, d], x_sb[:, d, t0:t0 + tsz],
                                             start=(d == 0), stop=(d == DD - 1))
                        for d in range(DD):
                            nc.tensor.matmul(pu, wu_sb[:, hh, d], x_sb[:, d, t0:t0 + tsz],
                                             start=(d == 0), stop=(d == DD - 1))
                        sw = work.tile([P, 512], f32, tag="sw", name="sw",
                                       bufs=4)[:, :tsz]
                        nc.scalar.activation(sw, pg, SIG)  # silu(gate)
                        nc.vector.tensor_mul(hT[:, hh, t0:t0 + tsz], sw, pu)

            def down_phase(n_h, wd_sb, hT, out_d, toks, bc_sb, outq):
                for (t0, tsz) in toks:
                    for o in range(DD):
                        pd = pp()[:, :tsz]
                        for j in range(n_h):
                            nc.tensor.matmul(pd, wd_sb[:, o, j], hT[:, j, t0:t0 + tsz],
                                             start=(j == 0), stop=(j == n_h - 1))
                        yt = work.tile([P, 512], bf16, tag="yt", name="yt",
                                       bufs=32)[:, :tsz]
                        if bc_sb is not None:
                            nc.vector.tensor_mul(yt, pd, bc_sb[:, t0:t0 + tsz])
                        else:
                            nc.vector.tensor_copy(yt, pd)
                        outq.dma_start(out_d[o, :, t0:t0 + tsz], yt)

            gu_phase(HE, wg_sbs[0], wu_sbs[0], xg_sbs[0], hT_sbs[0], tiless[0])
            gu_phase(HE, wg_sbs[1], wu_sbs[1], xg_sbs[1], hT_sbs[1], tiless[1])
            down_phase(HE, wd_sbs[0], hT_sbs[0], ye_ds[0], tiless[0],
                       bc_sbs[0], nc.sync)
            down_phase(HE, wd_sbs[1], hT_sbs[1], ye_ds[1], tiless[1],
                       bc_sbs[1], nc.scalar)
            gu_phase(HS, wgs_sb, wus_sb, xs_sb, sT_sb, [(0, TS)])
            down_phase(HS, sd_sb, sT_sb, ys_d, [(0, TS)], None, nc.scalar)

    nc.compile()
    return nc


def kernel(x, router_w, router_bias, up_proj, gate_proj, down_proj,
           sh_gate, sh_up, sh_down):
    global LAST_RESULT
    from concourse.bass_utils import run_bass_kernel_spmd

    x = np.asarray(x, np.float32)
    B, T, D = x.shape
    N = B * T
    flat = np.ascontiguousarray(x.reshape(N, D))

    # ---- host router (fp64 for a stable top-k; margins >> fp32 noise) ----
    logits = flat.astype(np.float64) @ np.asarray(router_w, np.float64).T \
        + np.asarray(router_bias, np.float64)
    top2 = np.argpartition(-logits, TOPK - 1, axis=1)[:, :TOPK]
    lsel = np.take_along_axis(logits, top2, axis=1)
    lsel -= lsel.max(axis=1, keepdims=True)
    sc = np.exp(lsel)
    sc /= sc.sum(axis=1, keepdims=True)          # [N, 2] combine weights (fp64)

    tok_idx, tok_w = [], []
    for e in range(E):
        rows, slots = np.nonzero(top2 == e)
        tok_idx.append(rows)
        tok_w.append(sc[rows, slots].astype(np.float32))
    cnts = np.array([len(i) for i in tok_idx])
    # load-balance: the 8 busiest experts go to slot 0, the rest to slot 1,
    # so slot 1 gets a smaller capacity (less padded compute).
    order = np.argsort(-cnts, kind="stable")
    slot_experts = [order[:N_CORES], order[N_CORES:]]   # [slot][core] -> expert

    def _cap(mx):
        return max(256, 16 * math.ceil(mx / 16))

    C0 = _cap(max(cnts[e] for e in slot_experts[0]))
    C1 = _cap(max(cnts[e] for e in slot_experts[1]))
    if C1 > C0:
        C0 = C1
    CS = (C0, C1)

    if CS not in _PROGRAMS:
        _PROGRAMS[CS] = _build_program(C0, C1)
    nc = _PROGRAMS[CS]

    # ---- build per-core inputs ----
    flatT = np.ascontiguousarray(flat.T)          # [D, N]

    def gu_pack(w_in_out):                        # [D, H] -> [128, H/128, D/128, 128]
        Din, H = w_in_out.shape
        return np.ascontiguousarray(
            w_in_out.reshape(Din // P, P, H // P, P).transpose(1, 2, 0, 3)
        ).astype(_BF)

    wgsT = gu_pack(np.asarray(sh_gate, np.float32).T)
    wusT = gu_pack(np.asarray(sh_up, np.float32).T)
    sdT = gu_pack(np.asarray(sh_down, np.float32).T)

    in_maps = []
    for c in range(N_CORES):
        m = {"xs": np.ascontiguousarray(
            flatT[:, TS * c:TS * (c + 1)].reshape(D // P, P, TS).transpose(1, 0, 2)
        ).astype(_BF), "wgs": wgsT, "wus": wusT, "sd": sdT}
        for j in range(2):
            e = int(slot_experts[j][c])
            Cj = CS[j]
            idx, w = tok_idx[e], tok_w[e]
            xg = np.zeros((P, D // P, Cj), _BF)
            bc = np.zeros((P, Cj), _BF)
            g = flatT[:, idx]                     # [D, cnt]
            xg[:, :, :len(idx)] = g.reshape(D // P, P, len(idx)).transpose(1, 0, 2).astype(_BF)
            bc[:, :len(idx)] = w[None, :].astype(_BF)
            m[f"xg{j}"] = xg
            m[f"bc{j}"] = bc
            m[f"wg{j}"] = gu_pack(np.asarray(gate_proj[e], np.float32))
            m[f"wu{j}"] = gu_pack(np.asarray(up_proj[e], np.float32))
            m[f"wd{j}"] = gu_pack(np.asarray(down_proj[e], np.float32))
        in_maps.append(m)

    try:
        res = run_bass_kernel_spmd(nc, in_maps, core_ids=list(range(N_CORES)))
    except Exception:
        res = run_bass_kernel_spmd(nc, in_maps, core_ids=list(range(N_CORES)))
    LAST_RESULT = res

    # ---- unshard: scatter-add expert outputs, add shared shard ----
    y = np.zeros((N, D), np.float32)
    for c in range(N_CORES):
        for j in range(2):
            e = int(slot_experts[j][c])
            idx = tok_idx[e]
            ye = np.asarray(res.results[c][f"ye{j}"], dtype=np.float32)
            y[idx] += ye.reshape(D, CS[j])[:, :len(idx)].T
        ys = np.asarray(res.results[c]["ys"], dtype=np.float32).reshape(D, TS)
        y[TS * c:TS * (c + 1)] += ys.T
    return y.reshape(B, T, D)
